# revision 11
# baseline (speedup 1.0000x reference)
"""Trainium2 Bass kernel for GNN message passing (nn_Kernel_17772574670927).

Reference computes, per node b with N=8 neighbors:
    out[b] = sum_n concat(node_v[b], node_h[b], nbr_v[b,n], nbr_h[b,n]) @ W + bias
Since the linear layer distributes over the neighbor sum:
    out[b] = (N*node_v[b])*W[0] + (N*node_h[b]) @ W[1:257]
           + (sum_n nbr_v[b,n])*W[257] + (sum_n nbr_h[b,n]) @ W[258:514] + N*bias
an 8x FLOP reduction vs the naive form; the kernel is then HBM-bound on
streaming nbr_h (512 MB total).

Layout/engine plan (v3):
  - gpsimd SWDGE: the nbr_h stream with fp32->bf16 cast during DMA --
    1-tile (1 MiB read) granules, 12 buffered (6 MiB SBUF).  The cast
    halves SBUF-side writes and feeds an all-bf16 DVE add tree at 2x
    rate (0.93us/tile vs 2.1 for fp32), decoupling compute from the
    read pace.
  - sync HWDGE ring: node_h chunks (fp32, 2 MiB, 3 buffered).
  - scalar HWDGE ring: three small setup reads + per-tile bf16 output
    writes (no always-ready reads left on this ring to block).
  - neighbor sum: 3 contiguous bf16 DVE add levels.
  - nbr-sum PE transposes in bf16; node_h transposes fp32 into bf16
    xt copies.
  - tiny tensors (node_v, sum_n nbr_v, W row/bias scaling, weight-block
    layout, bf16 casts of the weights) are prepared host-side in numpy
    and shipped as three small pre-arranged bf16 inputs:
      wb  [4, 128, H]: N*W[1:129], N*W[129:257], W[258:386], W[386:514]
      vw  [3, H]:      N*W[0], N*b, W[257]
      nv3 [3, BP]:     node_v, ones, sum_n nbr_v   (K=3 lhsT columns)

Sharding: data-parallel over the node dim B=65536 across 8 cores (8192 each).
Weights replicated. No collectives.
"""
import sys

for _p in ("/root/.axon_site", "/root/.axon_site/_ro/trn_rl_repo", "/opt/trn_rl_repo"):
    if _p not in sys.path:
        sys.path.append(_p)

import ml_dtypes
import numpy as np

import concourse.bacc as bacc
import concourse.bass as bass
import concourse.mybir as mybir
from concourse.masks import make_identity
from concourse.tile import TileContext

B, N, H = 65536, 8, 256
NCORES = 8
BP = B // NCORES          # 8192 nodes per core
P = 128                   # SBUF partitions
NTILES = BP // P          # 64 node-tiles per core
CHUNK = 8                 # tiles per node_h chunk (2 MiB fp32 in SBUF)
NCHUNKS = NTILES // CHUNK
F32 = mybir.dt.float32
BF16 = mybir.dt.bfloat16
BF16NP = ml_dtypes.bfloat16


def build_bass() -> bass.Bass:
    nc = bacc.Bacc("TRN2", target_bir_lowering=False, debug=False,
                   num_devices=NCORES)
    node_h = nc.dram_tensor("node_h", [BP, H], F32, kind="ExternalInput")
    nbr_h = nc.dram_tensor("nbr_h", [BP, N, H], F32, kind="ExternalInput")
    wb = nc.dram_tensor("wb", [4, P, H], BF16, kind="ExternalInput")
    vw = nc.dram_tensor("vw", [3, H], BF16, kind="ExternalInput")
    nv3 = nc.dram_tensor("nv3", [3, BP], BF16, kind="ExternalInput")
    # bf16 output halves the write traffic; host upcasts to fp32 after gather
    out = nc.dram_tensor("out", [BP, H], BF16, kind="ExternalOutput")

    with TileContext(nc) as tc, nc.allow_low_precision(
        reason="bf16 GEMM inputs; harness tolerance is 2e-2"
    ):
        with (
            tc.tile_pool(name="singles", bufs=1) as singles,
            tc.tile_pool(name="nbr", bufs=12) as nbr_pool,
            tc.tile_pool(name="a1", bufs=3) as a1_pool,
            tc.tile_pool(name="a2", bufs=3) as a2_pool,
            tc.tile_pool(name="a3", bufs=3) as a3_pool,
            tc.tile_pool(name="nodeh", bufs=3) as nodeh_pool,
            tc.tile_pool(name="outp", bufs=4) as out_pool,
            tc.tile_pool(name="xt", bufs=4) as xt_pool,
            tc.tile_pool(name="ptf", bufs=2, space="PSUM") as ptf_pool,
            tc.tile_pool(name="ptb", bufs=2, space="PSUM") as ptb_pool,
            tc.tile_pool(name="pout", bufs=3, space="PSUM") as psum_out_pool,
        ):
            # ---- nbr_h stream: gpsimd SWDGE with fp32->bf16 cast, one
            # tile per granule.  The gpsimd sequencer owns nothing else;
            # it runs ahead issuing reads until the pool back-pressures.
            nbr_tiles = []
            for t in range(NTILES):
                nb = nbr_pool.tile([P, N, H], BF16, tag="nbr")
                nc.gpsimd.dma_start(
                    out=nb[:], in_=nbr_h[t * P:(t + 1) * P, :, :]
                )
                nbr_tiles.append(nb)

            # ---- node_h chunks: plain fp32 loads on the sync ring
            # (always-ready, so they interleave with nothing blocking)
            nodeh_chunks = []
            for c in range(NCHUNKS):
                ph = nodeh_pool.tile([P, CHUNK, H], F32, tag="nodeh")
                nc.sync.dma_start(
                    out=ph[:],
                    in_=node_h[c * CHUNK * P:(c + 1) * CHUNK * P, :].rearrange(
                        "(t p) h -> p t h", p=P
                    ),
                )
                nodeh_chunks.append(ph)

            # ---- one-time setup: three small bf16 loads on the scalar ring
            w_sb = singles.tile([P, 4, H], BF16)
            nc.scalar.dma_start(
                out=w_sb[:], in_=wb.rearrange("c p h -> p c h")
            )
            v_w3 = singles.tile([3, H], BF16)
            nc.scalar.dma_start(out=v_w3[:], in_=vw[:, :])
            vcolsT = singles.tile([3, NTILES, P], BF16)
            nc.scalar.dma_start(
                out=vcolsT[:], in_=nv3.rearrange("c (t p) -> c t p", p=P)
            )

            identity = singles.tile([P, P], F32)
            make_identity(nc, identity)
            identity_bf = singles.tile([P, P], BF16)
            nc.scalar.copy(out=identity_bf[:], in_=identity[:])

            # ---- main loop ----
            for t in range(NTILES):
                nbr_tile = nbr_tiles[t]
                c, j = divmod(t, CHUNK)
                nodeh_chunk = nodeh_chunks[c]

                if True:
                    # neighbor sum: 3-level contiguous bf16 add tree on DVE
                    a1 = a1_pool.tile([P, 4, H], BF16)
                    nc.vector.tensor_add(
                        out=a1[:], in0=nbr_tile[:, 0:4, :], in1=nbr_tile[:, 4:8, :]
                    )
                    a2 = a2_pool.tile([P, 2, H], BF16)
                    nc.vector.tensor_add(
                        out=a2[:], in0=a1[:, 0:2, :], in1=a1[:, 2:4, :]
                    )
                    a3 = a3_pool.tile([P, H], BF16)
                    nc.vector.tensor_add(
                        out=a3[:], in0=a2[:, 0, :], in1=a2[:, 1, :]
                    )

                    # transpose the four 128-feature blocks to feature-major
                    xt = xt_pool.tile([P, 4, P], BF16)
                    for i in range(2):
                        ptf = ptf_pool.tile([P, P], F32, tag="ptf")
                        nc.tensor.transpose(
                            ptf[:], nodeh_chunk[:, j, i * 128:(i + 1) * 128],
                            identity[:],
                        )
                        nc.scalar.copy(out=xt[:, i, :], in_=ptf[:])
                    for i in range(2):
                        ptb = ptb_pool.tile([P, P], BF16, tag="ptb")
                        nc.tensor.transpose(
                            ptb[:], a3[:, i * 128:(i + 1) * 128], identity_bf[:]
                        )
                        nc.scalar.copy(out=xt[:, 2 + i, :], in_=ptb[:])

                    # accumulate all five K-blocks into PSUM (bias included)
                    psum_out = psum_out_pool.tile([P, H], F32)
                    for i in range(4):
                        nc.tensor.matmul(
                            psum_out[:], xt[:, i, :], w_sb[:, i, :],
                            start=(i == 0), stop=False,
                        )
                    nc.tensor.matmul(
                        psum_out[:], vcolsT[:, t, :], v_w3[:],
                        start=False, stop=True,
                    )

                    # per-tile bf16 write on the scalar ring: the only other
                    # traffic there is one-time setup, so the data-dependent
                    # writes can't block any always-ready read stream
                    out_tile = out_pool.tile([P, H], BF16)
                    nc.scalar.copy(out=out_tile[:], in_=psum_out[:])
                    nc.scalar.dma_start(
                        out=out[t * P:(t + 1) * P, :], in_=out_tile[:]
                    )
    nc.compile()
    return nc


_BASS_CACHE = None


def _get_bass():
    global _BASS_CACHE
    if _BASS_CACHE is None:
        _BASS_CACHE = build_bass()
    return _BASS_CACHE


def _prep_small(inputs: dict):
    """Host-side prep of the tiny pre-arranged bf16 weight tensors."""
    Wf = np.asarray(inputs["W"], dtype=np.float32)
    bf = np.asarray(inputs["b"], dtype=np.float32)
    node_v = np.asarray(inputs["node_v"], dtype=np.float32).reshape(B)
    nbr_v = np.asarray(inputs["nbr_v"], dtype=np.float32).reshape(B, N)

    wb = np.empty((4, P, H), dtype=np.float32)
    wb[0] = N * Wf[1:129]
    wb[1] = N * Wf[129:257]
    wb[2] = Wf[258:386]
    wb[3] = Wf[386:514]
    vw = np.stack([N * Wf[0], N * bf, Wf[257]], axis=0)
    nv3 = np.stack(
        [node_v, np.ones(B, dtype=np.float32), nbr_v.sum(axis=1)], axis=0
    )
    return (
        np.ascontiguousarray(wb.astype(BF16NP)),
        np.ascontiguousarray(vw.astype(BF16NP)),
        np.ascontiguousarray(nv3.astype(BF16NP)),
    )


def run_sharded(inputs: dict, trace: bool = False, trace_cores=None):
    """Shard full inputs over 8 cores, run, gather. Returns (out, results)."""
    from concourse.bass_utils import run_bass_kernel_spmd

    nc = _get_bass()
    node_h = np.ascontiguousarray(np.asarray(inputs["node_h"], dtype=np.float32))
    nbr_h = np.ascontiguousarray(np.asarray(inputs["nbr_h"], dtype=np.float32))
    wb, vw, nv3 = _prep_small(inputs)

    in_maps = []
    for core in range(NCORES):
        s = slice(core * BP, (core + 1) * BP)
        in_maps.append({
            "node_h": node_h[s], "nbr_h": nbr_h[s],
            "wb": wb, "vw": vw,
            "nv3": np.ascontiguousarray(nv3[:, s]),
        })
    kwargs = {}
    if trace:
        kwargs.update(trace=True, trace_cores=trace_cores or [0])
    res = run_bass_kernel_spmd(nc, in_maps, core_ids=list(range(NCORES)), **kwargs)
    full = np.concatenate(
        [np.asarray(res.results[i]["out"]).astype(np.float32) for i in range(NCORES)],
        axis=0,
    )
    return full, res


def kernel(**inputs) -> np.ndarray:
    # Retry guards against the rare transient device error
    # (NRT_EXEC_UNIT_UNRECOVERABLE) seen on back-to-back runs; the compiled
    # NEFF is cached so a retry only re-executes.
    import time as _time

    last_err = None
    for attempt in range(3):
        try:
            out, _ = run_sharded(inputs, trace=False)
            return out
        except Exception as e:  # noqa: BLE001 - re-raised after retries
            last_err = e
            _time.sleep(2.0)
    raise last_err


if __name__ == "__main__":
    rng = np.random.default_rng(0)
    fake = {
        "node_v": rng.standard_normal((B, 1), dtype=np.float32),
        "node_h": rng.standard_normal((B, H), dtype=np.float32),
        "nbr_v": rng.standard_normal((B, N, 1), dtype=np.float32),
        "nbr_h": rng.standard_normal((B, N, H), dtype=np.float32),
        "W": rng.standard_normal((514, H), dtype=np.float32) / np.sqrt(514),
        "b": np.zeros((H,), dtype=np.float32),
        "iteration": 0,
    }
    got = kernel(**fake)
    sf = np.concatenate([fake["node_v"], fake["node_h"]], axis=-1)
    nf = np.concatenate([fake["nbr_v"], fake["nbr_h"]], axis=-1)
    exp = (
        N * sf @ fake["W"][:257] + nf.sum(axis=1) @ fake["W"][257:] + N * fake["b"]
    )
    err = np.abs(got - exp).max() / np.abs(exp).max()
    print("rel err vs numpy:", err)
